# revision 1
# baseline (speedup 1.0000x reference)
"""Trainium2 Bass kernel for nn_Decoder (GNN edge decoder).

Math: node MLP -> per-pair edge MLP -> symmetric adjacency.
Key rewrite: edge layer-1 concat(z_i, z_j) @ We1 == A_i + B_j with
  A = emb @ We1[:E] + be1,  B = emb @ We1[E:]
so the device streams contiguous triangle rows with a broadcast-add
instead of gathering P=32640 pair vectors.

Device layout (per core, uniform SPMD program, data shifted per core):
  - pairs processed as dual rows: segment m handles rows (16m+2k, 16m+2k+1)
    for core k; top/bottom 64 SBUF partitions hold the two rows.
  - Apk [128, NB]: top = A_T shifted by 2k nodes, bottom = further shifted
    by one node (so one broadcast AP feeds both rows).
  - mm2: blockdiag(We2, We2) [128,128] stationary, rhs = relu(pre).
  - mm3: lhsT = t2-subchunk (stationary), rhs = [[We3,0],[0,We3]] -> logits
    land partition-major, cheap PSUM->SBUF copy.
Host assembles the symmetric adjacency from per-core logit blocks.
"""

import sys

import numpy as np

if "/opt/trn_rl_repo" not in sys.path:
    sys.path.insert(0, "/opt/trn_rl_repo")

import ml_dtypes

B, LAT, ST, N, E, H = 64, 256, 32, 256, 32, 64
NB = N * B  # 16384 node-cols (node-major, b inner)
NSEG = 16  # segments per core (even rows 16m+2k)
CHUNK = 512
BF16 = ml_dtypes.bfloat16

_cache = {}


def _layout():
    """Uniform chunk enumeration shared by builder and assembler.

    Returns list of (m, c0, F): segment m covers local rows (16m, 16m+1),
    local j-blocks 16m+1 .. 255, i.e. ncols = (255-16m)*64; chunked by 512.
    """
    if "layout" in _cache:
        return _cache["layout"]
    chunks = []
    for m in range(NSEG):
        ncols = (255 - 16 * m) * B
        for c0 in range(0, ncols, CHUNK):
            chunks.append((m, c0, min(CHUNK, ncols - c0)))
    _cache["layout"] = chunks
    return chunks


def _n_chunks():
    return len(_layout())


def _build_nc():
    import concourse.bass as bass
    import concourse.mybir as mybir
    from concourse.tile import TileContext

    bf = mybir.dt.bfloat16
    f32 = mybir.dt.float32
    nc = bass.Bass()
    inp_d = nc.dram_tensor("inp", [128, 2 * NB + 130], bf, kind="ExternalInput")
    nch = _n_chunks()
    out_d = nc.dram_tensor("logits", [128, nch * 8], f32, kind="ExternalOutput")

    with TileContext(nc) as tc:
        with (
            tc.tile_pool(name="const", bufs=1) as cpool,
            tc.tile_pool(name="work", bufs=4) as wpool,
            tc.tile_pool(name="out", bufs=1) as opool,
            tc.tile_pool(name="ps2", bufs=4, space="PSUM") as ps2pool,
            tc.tile_pool(name="ps3", bufs=3, space="PSUM") as ps3pool,
        ):
            inp = cpool.tile([128, 2 * NB + 130], bf, tag="inp")
            nc.sync.dma_start(inp[:], inp_d[:])
            apk = inp[:, 0:NB]
            bpk = inp[:, NB : 2 * NB]
            w2 = inp[:, 2 * NB : 2 * NB + 128]
            w3 = inp[:, 2 * NB + 128 : 2 * NB + 130]
            logits_sb = opool.tile([128, nch * 8], f32, tag="lg")

            # Absorb the many HW-DGE queue-semaphore waits of the big input
            # DMAs on plain copy instructions; the broadcast tensor_add's
            # 3D TensorTensor encoding has too few wait-command slots.
            probe = cpool.tile([128, 8], bf, tag="probe")
            nc.vector.tensor_copy(probe[:, 0:2], inp[:, 0:2])
            psum_probe = ps3pool.tile([128, 8], f32, tag="ps3")
            nc.tensor.matmul(
                psum_probe[:2, :2], inp[:, 0:2], inp[:, 2:4], start=True, stop=True
            )

            for ci, (m, c0, F) in enumerate(_layout()):
                abase = 16 * m * B  # A-block col of local row 16m
                cbase = (16 * m + 1) * B + c0  # B cols for this chunk
                pre = wpool.tile([128, CHUNK], bf, tag="pre")
                t2 = wpool.tile([128, CHUNK], bf, tag="t2")
                # broadcast AP: repeat A block (64 cols) F//64 times
                a_blk = inp[:, abase : abase + B]
                a_bc = bass.AP(
                    a_blk.tensor,
                    a_blk.offset,
                    [list(a_blk.ap[0]), [0, F // B], [1, B]],
                )
                b_sl = inp[:, NB + cbase : NB + cbase + F]
                nc.vector.tensor_add(pre[:, :F], b_sl, a_bc)
                nc.gpsimd.tensor_relu(pre[:, :F], pre[:, :F])
                psum2 = ps2pool.tile([128, CHUNK], f32, tag="ps2")
                nc.tensor.matmul(
                    psum2[:, :F], w2, pre[:, :F], start=True, stop=True
                )
                nc.scalar.activation(
                    t2[:, :F],
                    psum2[:, :F],
                    mybir.ActivationFunctionType.Relu,
                )
                psum3 = ps3pool.tile([128, 8], f32, tag="ps3")
                for sc in range((F + 127) // 128):
                    M = min(128, F - sc * 128)
                    nc.tensor.matmul(
                        psum3[:M, sc * 2 : sc * 2 + 2],
                        t2[:, sc * 128 : sc * 128 + M],
                        w3,
                        start=True,
                        stop=True,
                    )
                nc.vector.tensor_copy(
                    logits_sb[:, ci * 8 : ci * 8 + 8], psum3[:]
                )
            nc.sync.dma_start(out_d[:], logits_sb[:])

    raw = nc.to_json_bytes()
    legal = _legalize_sync(raw)
    nc.to_json_bytes = lambda: legal
    return nc


def _legalize_sync(bir_bytes):
    """Split multi-wait sync_info into single-wait EventSemaphore preludes.

    The walrus build in this container encodes at most one sync-wait command
    per instruction for several ISA structs; Tile emits up to ~9 on the tail
    drain. Semantics are preserved: waits execute in order on the same engine
    ahead of the original instruction.
    """
    import json as _json

    bir = _json.loads(bir_bytes)
    for f in bir["functions"]:
        ctr = [0]
        # template EventSemaphore per engine (from the tail barrier)
        templates = {}
        for blk in f["blocks"]:
            for ins in blk.get("instructions") or []:
                if ins.get("opcode") == "EventSemaphore":
                    templates.setdefault(ins.get("engine"), ins)
        for blk in f["blocks"]:
            insts = blk.get("instructions")
            if not insts:
                continue
            out = []
            for ins in insts:
                si = ins.get("sync_info") or {}
                waits = si.get("on_wait") or []
                keep = 0 if ins.get("opcode") == "TensorTensor" else 1
                if len(waits) > keep:
                    tpl = templates.get(ins.get("engine"))
                    if tpl is not None:
                        moved = waits[: len(waits) - keep]
                        for w in moved:
                            ctr[0] += 1
                            nw = _json.loads(_json.dumps(tpl))
                            nw["name"] = f"escw_{ctr[0]}"
                            nw["sync_info"] = {"on_update": [], "on_wait": [w]}
                            out.append(nw)
                        si["on_wait"] = waits[len(waits) - keep :]
                out.append(ins)
            blk["instructions"] = out
    return _json.dumps(bir).encode()


def _host_prep(latent_z, stats, W1, b1, W2, b2, We1, be1, We2, be2, We3, be3):
    """Node MLP + A/B decomposition on host (0.5% of total FLOPs)."""
    x = np.concatenate([latent_z, stats], axis=-1).astype(np.float32)
    h = np.maximum(x @ W1 + b1, 0.0)
    emb = (h @ W2 + b2).reshape(B, N, E)
    A = emb @ We1[:E] + be1  # [B, N, H]
    Bm = emb @ We1[E:]  # [B, N, H]
    # node-major transposed: [H, N*B], col = n*B + b
    A_T = np.ascontiguousarray(A.transpose(2, 1, 0).reshape(H, NB))
    B_T = np.ascontiguousarray(Bm.transpose(2, 1, 0).reshape(H, NB))
    w2blk = np.zeros((128, 128), np.float32)
    w2blk[:H, :H] = We2
    w2blk[H:, H:] = We2
    w3sep = np.zeros((128, 2), np.float32)
    w3sep[:H, 0] = We3[:, 0]
    w3sep[H:, 1] = We3[:, 0]
    return A_T, B_T, w2blk, w3sep, be3


def _shifted(T, sh):
    """[64, NB] -> [64, NB] shifted left by sh cols, zero-padded."""
    out = np.zeros((H, NB), np.float32)
    if sh < NB:
        out[:, : NB - sh] = T[:, sh:]
    return out


def _assembly_indices():
    """Per-element mapping of logits_sb[p, col] -> (b, i_loc, j_loc, g)."""
    if "asm" in _cache:
        return _cache["asm"]
    rows, cols, bs, ilocs, jlocs = [], [], [], [], []
    for ci, (m, c0, F) in enumerate(_layout()):
        for sc in range((F + 127) // 128):
            M = min(128, F - sc * 128)
            p = np.arange(M)
            c = c0 + sc * 128 + p  # local col within segment
            jb = 16 * m + 1 + c // B
            b = c % B
            for g in (0, 1):
                rows.append(p)
                cols.append(np.full(M, ci * 8 + sc * 2 + g))
                bs.append(b)
                ilocs.append(np.full(M, 16 * m + g))
                jlocs.append(jb)
    out = tuple(
        np.concatenate(a) for a in (rows, cols, bs, ilocs, jlocs)
    )
    _cache["asm"] = out
    return out


def kernel(**inputs):
    from concourse.bass_utils import run_bass_kernel_spmd

    inp = {k: np.asarray(v, np.float32) for k, v in inputs.items()}
    A_T, B_T, w2blk, w3sep, be3 = _host_prep(**inp)

    in_maps = []
    for k in range(8):
        sh = 2 * k * B
        apk = np.empty((128, NB), np.float32)
        apk[:H] = _shifted(A_T, sh)
        apk[H:] = _shifted(A_T, sh + B)
        bpk = np.empty((128, NB), np.float32)
        bpk[:H] = bpk[H:] = _shifted(B_T, sh)
        in_maps.append(
            {
                "inp": np.ascontiguousarray(
                    np.concatenate(
                        [apk, bpk, np.concatenate([w2blk, w3sep], 1)], axis=1
                    ).astype(BF16)
                )
            }
        )

    import time as _time
    nc = _cache.get("nc")
    if nc is None:
        nc = _build_nc()
        _cache["nc"] = nc
    t0 = _time.time()
    res = run_bass_kernel_spmd(nc, in_maps, core_ids=list(range(8)))
    globals()["last_results"] = res
    globals()["last_run_s"] = _time.time() - t0

    rows, cols, bs, ilocs, jlocs = _assembly_indices()
    adj = np.zeros((B, N, N), np.float32)
    for k in range(8):
        lg = np.asarray(res.results[k]["logits"], np.float32)
        i = ilocs + 2 * k
        j = jlocs + 2 * k
        valid = (j < N) & (j > i)
        v = lg[rows[valid], cols[valid]] + float(be3[0])
        ii, jj, bb = i[valid], j[valid], bs[valid]
        adj[bb, ii, jj] = v
        adj[bb, jj, ii] = v
    return adj



# revision 9
# speedup vs baseline: 1.4861x; 1.4861x over previous
"""Trainium2 Bass kernel for nn_Decoder (GNN edge decoder).

Math: node MLP -> per-pair edge MLP -> symmetric adjacency.

Key rewrites vs the naive pair loop:
  1. Edge layer-1: concat(z_i, z_j) @ We1 == A_i + B_j with
       A = emb @ We1[:E] + be1,  B = emb @ We1[E:].
  2. relu(A+B) == max(A, -B) + B. The max is ONE vector op (no separate
     relu), and the +B term is linear so it folds into layer-2 as a second
     accumulating matmul with stationary -We2 and rhs -B (PSUM does the add).
     A tunable fraction of groups (LAM_N) instead materialize t1 = relu(A+B)
     with add (DVE) + relu (GPSIMD) and use a single matmul, trading PE time
     against vector-engine time.
  3. Work is batched in 1536-col groups (3 PSUM banks): one pair-op, 3x1-2
     matmuls, one batched relu2 (ACT with fused be2 bias / DVE tensor_scalar
     add+max), 12 tiny mm3 matmuls (stationary = t2 subchunk, rhs = w3 -> 2
     cols out, LoadStationary is free), logits packed into one PSUM bank per
     21 groups then copied+DMA'd out once.

Device layout (per core, uniform SPMD program, data shifted per core):
  segment s = row pair (16s+2k, 16s+2k+1) for core k; j-window
  [16s+1+2k, 255+2k] streamed as contiguous cols of the 2k-shifted node
  tensor (j > 255 region is zero padding, filtered on host).
Host assembles the symmetric adjacency from per-core logit blocks.
"""

import sys

import numpy as np

if "/opt/trn_rl_repo" not in sys.path:
    sys.path.insert(0, "/opt/trn_rl_repo")

import ml_dtypes

B, LAT, ST, N, E, H = 64, 256, 32, 256, 32, 64
NB = N * B  # 16384 node-major cols (col = n*64 + b)
NSEG = 16
GRP = 1536  # group cols (3 PSUM banks)
SLOT = 24  # psum3 cols per group
SBG = 21  # groups per psum3 bank (superblock)
# input tile column layout
APK_O = NB
W2_O = APK_O + NSEG * B  # 17408
W2N_O = W2_O + 128
W3_O = W2N_O + 128
BE2_O = W3_O + 2
INPC = BE2_O + 1  # 17667

# --- engine assignment tuning ---------------------------------------------
# Pool supports only tensor+scalar encodings (no TensorTensor/STT), so the
# pair op is DVE-only; Pool takes relu1 of ARCH-1 groups.
LAM_N = 14  # groups using add+relu1 + single matmul (ARCH-1)
ACT_N = 79  # relu2 on ACT (rest on DVE)

BF16 = ml_dtypes.bfloat16

_cache = {}


def _groups():
    """(g, s, c0, gF) for each group; segment s window = (255-16s) blocks."""
    if "groups" in _cache:
        return _cache["groups"]
    out = []
    g = 0
    for s in range(NSEG):
        ncols = (255 - 16 * s) * B
        for c0 in range(0, ncols, GRP):
            out.append((g, s, c0, min(GRP, ncols - c0)))
            g += 1
    _cache["groups"] = out
    return out


def _ngrp():
    return len(_groups())


def _spread(n, total):
    """n indices evenly spread in range(total)."""
    if n <= 0:
        return set()
    return {int(i * total / n + total / (2 * n)) for i in range(n)}


def _assign():
    """Per-group (arch1_mode, relu2_on_act) assignment."""
    if "assign" in _cache:
        return _cache["assign"]
    ngrp = _ngrp()
    arch1 = _spread(LAM_N, ngrp)
    act = _spread(ACT_N, ngrp)
    out = [(g in arch1, g in act) for g in range(ngrp)]
    _cache["assign"] = out
    return out


def _build_nc():
    import concourse.bass as bass
    import concourse.mybir as mybir
    from concourse.tile import TileContext

    bf = mybir.dt.bfloat16
    f32 = mybir.dt.float32
    Relu = mybir.ActivationFunctionType.Relu
    Alu = mybir.AluOpType
    nc = bass.Bass()
    ngrp = _ngrp()
    nsb = (ngrp + SBG - 1) // SBG
    lout = nsb * SBG * SLOT
    inp_d = nc.dram_tensor("inp", [128, INPC], bf, kind="ExternalInput")
    out_d = nc.dram_tensor("logits", [128, lout], f32, kind="ExternalOutput")
    assign = _assign()

    with TileContext(nc) as tc:
        with (
            tc.tile_pool(name="const", bufs=1) as cpool,
            tc.tile_pool(name="mw", bufs=4) as mpool,
            tc.tile_pool(name="tw", bufs=4) as tpool,
            tc.tile_pool(name="ps2", bufs=2, space="PSUM") as ps2pool,
            tc.tile_pool(name="ps3", bufs=2, space="PSUM") as ps3pool,
        ):
            inp = cpool.tile([128, INPC], bf, tag="inp")
            # staged input DMA in consumption order: weights+apk first, then
            # bneg pieces, so early groups start while the rest streams in.
            nc.sync.dma_start(inp[:, APK_O:INPC], inp_d[:, APK_O:INPC])
            NPC = 8
            per = ((NB + NPC - 1) // NPC + 63) // 64 * 64
            for pc in range(NPC):
                lo, hi = pc * per, min(NB, (pc + 1) * per)
                if lo < hi:
                    nc.sync.dma_start(inp[:, lo:hi], inp_d[:, lo:hi])
            bneg = inp[:, 0:NB]
            w2 = inp[:, W2_O : W2_O + 128]
            w2n = inp[:, W2N_O : W2N_O + 128]
            w3 = inp[:, W3_O : W3_O + 2]
            be2f = cpool.tile([128, 1], f32, tag="be2f")
            nc.vector.tensor_copy(be2f[:], inp[:, BE2_O : BE2_O + 1])
            be2 = be2f[:]

            # Absorb the HW-DGE queue-semaphore waits of the input DMAs on
            # plain copy instructions (TensorTensor's 3D encoding has too few
            # wait-command slots; see _legalize_sync).
            probe = cpool.tile([128, 8], bf, tag="probe")
            nc.vector.tensor_copy(probe[:, 0:2], inp[:, 0:2])
            nc.gpsimd.tensor_copy(probe[:, 2:4], inp[:, 0:2])
            ps_probe = ps2pool.tile([128, GRP], f32, tag="ps2")
            nc.tensor.matmul(
                ps_probe[:2, :2], inp[:, 0:2], inp[:, 2:4], start=True, stop=True
            )
            nc.scalar.activation(probe[:2, 4:6], ps_probe[:2, :2], Relu)

            ps3 = None
            for g, s, c0, gF in _groups():
                arch1, on_act = assign[g]
                base = (16 * s + 1) * B + c0
                nblk = gF // B
                a_blk = inp[:, APK_O + s * B : APK_O + (s + 1) * B]
                a_bc = bass.AP(
                    a_blk.tensor,
                    a_blk.offset,
                    [list(a_blk.ap[0]), [0, nblk], [1, B]],
                )
                b_sl = inp[:, base : base + gF]
                m1 = mpool.tile([128, GRP], bf, tag="m1")
                if arch1:
                    # t1 = relu(A + B) = relu(A - (-B)); relu on Pool
                    nc.vector.tensor_sub(m1[:, :gF], a_bc, b_sl)
                    t1 = tpool.tile([128, GRP], bf, tag="t1")
                    nc.gpsimd.tensor_scalar_max(t1[:, :gF], m1[:, :gF], 0.0)
                    rhs1 = t1
                else:
                    # m1 = max(A, -B); +B folds into the second matmul
                    nc.vector.tensor_max(m1[:, :gF], b_sl, a_bc)
                    rhs1 = m1
                ps2 = ps2pool.tile([128, GRP], f32, tag="ps2")
                for ci in range((gF + 511) // 512):
                    o = ci * 512
                    F = min(512, gF - o)
                    if arch1:
                        nc.tensor.matmul(
                            ps2[:, o : o + F],
                            w2,
                            rhs1[:, o : o + F],
                            start=True,
                            stop=True,
                        )
                    else:
                        nc.tensor.matmul(
                            ps2[:, o : o + F],
                            w2,
                            rhs1[:, o : o + F],
                            start=True,
                            stop=False,
                        )
                        nc.tensor.matmul(
                            ps2[:, o : o + F],
                            w2n,
                            b_sl[:, o : o + F],
                            start=False,
                            stop=True,
                        )
                t2 = tpool.tile([128, GRP], bf, tag="t2")
                if on_act:
                    nc.scalar.activation(t2[:, :gF], ps2[:, :gF], Relu, bias=be2)
                else:
                    nc.vector.tensor_scalar(
                        t2[:, :gF], ps2[:, :gF], be2, 0.0, Alu.add, Alu.max
                    )
                sb, slot = divmod(g, SBG)
                if slot == 0:
                    ps3 = ps3pool.tile([128, SBG * SLOT], f32, tag="ps3")
                for sc in range((gF + 127) // 128):
                    M = min(128, gF - sc * 128)
                    nc.tensor.matmul(
                        ps3[:M, slot * SLOT + 2 * sc : slot * SLOT + 2 * sc + 2],
                        t2[:, sc * 128 : sc * 128 + M],
                        w3,
                        start=True,
                        stop=True,
                    )
                if slot == SBG - 1 or g == ngrp - 1:
                    lsb = tpool.tile([128, SBG * SLOT], f32, tag="lsb")
                    nc.vector.tensor_copy(lsb[:], ps3[:])
                    nc.sync.dma_start(
                        out_d[:, sb * SBG * SLOT : (sb + 1) * SBG * SLOT], lsb[:]
                    )

    raw = nc.to_json_bytes()
    legal = _legalize_sync(raw)
    nc.to_json_bytes = lambda: legal
    return nc


def _legalize_sync(bir_bytes):
    """Split multi-wait sync_info into single-wait EventSemaphore preludes.

    The walrus build in this container encodes at most one sync-wait command
    per instruction for several ISA structs; Tile emits up to ~9 on the tail
    drain. Semantics are preserved: waits execute in order on the same engine
    ahead of the original instruction.
    """
    import json as _json

    bir = _json.loads(bir_bytes)
    for f in bir["functions"]:
        ctr = [0]
        templates = {}
        for blk in f["blocks"]:
            for ins in blk.get("instructions") or []:
                if ins.get("opcode") == "EventSemaphore":
                    templates.setdefault(ins.get("engine"), ins)
        for blk in f["blocks"]:
            insts = blk.get("instructions")
            if not insts:
                continue
            out = []
            for ins in insts:
                si = ins.get("sync_info") or {}
                waits = si.get("on_wait") or []
                keep = 0 if ins.get("opcode") == "TensorTensor" else 1
                if len(waits) > keep:
                    tpl = templates.get(ins.get("engine"))
                    if tpl is not None:
                        moved = waits[: len(waits) - keep]
                        for w in moved:
                            ctr[0] += 1
                            nw = _json.loads(_json.dumps(tpl))
                            nw["name"] = f"escw_{ctr[0]}"
                            nw["sync_info"] = {"on_update": [], "on_wait": [w]}
                            out.append(nw)
                        si["on_wait"] = waits[len(waits) - keep :]
                out.append(ins)
            blk["instructions"] = out
    return _json.dumps(bir).encode()


def _host_prep(latent_z, stats, W1, b1, W2, b2, We1, be1, We2, be2, We3, be3):
    """Node MLP + A/B decomposition on host (0.5% of total FLOPs)."""
    x = np.concatenate([latent_z, stats], axis=-1).astype(np.float32)
    h = np.maximum(x @ W1 + b1, 0.0)
    emb = (h @ W2 + b2).reshape(B, N, E)
    A = emb @ We1[:E] + be1  # [B, N, H]
    Bm = emb @ We1[E:]  # [B, N, H]
    A_T = np.ascontiguousarray(A.transpose(2, 1, 0).reshape(H, NB))
    Bneg_T = np.ascontiguousarray((-Bm).transpose(2, 1, 0).reshape(H, NB))
    w2blk = np.zeros((128, 128), np.float32)
    w2blk[:H, :H] = We2
    w2blk[H:, H:] = We2
    w3sep = np.zeros((128, 2), np.float32)
    w3sep[:H, 0] = We3[:, 0]
    w3sep[H:, 1] = We3[:, 0]
    be2col = np.concatenate([be2, be2]).reshape(128, 1).astype(np.float32)
    return A_T, Bneg_T, w2blk, w3sep, be2col, be3


def _shifted(T, sh):
    """[64, NB] -> [64, NB] shifted left by sh cols, zero-padded."""
    out = np.zeros((H, NB), np.float32)
    if sh < NB:
        out[:, : NB - sh] = T[:, sh:]
    return out


def _core_input(A_T, Bneg_T, w2blk, w3sep, be2col, k):
    inp = np.zeros((128, INPC), np.float32)
    bsh = _shifted(Bneg_T, 2 * k * B)
    inp[:H, :NB] = bsh
    inp[H:, :NB] = bsh
    for s in range(NSEG):
        i = 16 * s + 2 * k
        inp[:H, APK_O + s * B : APK_O + (s + 1) * B] = A_T[:, i * B : (i + 1) * B]
        if i + 2 <= N:
            inp[H:, APK_O + s * B : APK_O + (s + 1) * B] = A_T[
                :, (i + 1) * B : (i + 2) * B
            ]
    inp[:, W2_O : W2_O + 128] = w2blk
    inp[:, W2N_O : W2N_O + 128] = -w2blk
    inp[:, W3_O : W3_O + 2] = w3sep
    inp[:, BE2_O : BE2_O + 1] = be2col
    return np.ascontiguousarray(inp.astype(BF16))


def _assembly_indices(k):
    """Vectorized mapping logits[m, col] -> (b, i, j) for core k."""
    key = ("asm", k)
    if key in _cache:
        return _cache[key]
    rows, cols, bs, iis, jjs = [], [], [], [], []
    for g, s, c0, gF in _groups():
        sb, slot = divmod(g, SBG)
        i_t = 16 * s + 2 * k
        for sc in range((gF + 127) // 128):
            M = min(128, gF - sc * 128)
            m = np.arange(M)
            cgl = c0 + sc * 128 + m
            cb = cgl // B  # window block: j = 16s+1+cb+2k
            b = cgl % B
            j = 16 * s + 1 + cb + 2 * k
            ocol = sb * SBG * SLOT + slot * SLOT + 2 * sc
            vt = j <= N - 1
            rows.append(m[vt])
            cols.append(np.full(vt.sum(), ocol))
            bs.append(b[vt])
            iis.append(np.full(vt.sum(), i_t))
            jjs.append(j[vt])
            vb = (j <= N - 1) & (cb >= 1)
            rows.append(m[vb])
            cols.append(np.full(vb.sum(), ocol + 1))
            bs.append(b[vb])
            iis.append(np.full(vb.sum(), i_t + 1))
            jjs.append(j[vb])
    out = tuple(np.concatenate(a) for a in (rows, cols, bs, iis, jjs))
    _cache[key] = out
    return out


def kernel(**inputs):
    from concourse.bass_utils import run_bass_kernel_spmd

    inp = {kk: np.asarray(v, np.float32) for kk, v in inputs.items()}
    A_T, Bneg_T, w2blk, w3sep, be2col, be3 = _host_prep(**inp)

    in_maps = [
        {"inp": _core_input(A_T, Bneg_T, w2blk, w3sep, be2col, k)}
        for k in range(8)
    ]

    import time as _time

    nc = _cache.get("nc")
    if nc is None:
        nc = _build_nc()
        _cache["nc"] = nc
    t0 = _time.time()
    res = run_bass_kernel_spmd(nc, in_maps, core_ids=list(range(8)))
    globals()["last_results"] = res
    globals()["last_run_s"] = _time.time() - t0

    adj = np.zeros((B, N, N), np.float32)
    for k in range(8):
        lg = np.asarray(res.results[k]["logits"], np.float32)
        rows, cols, bs, iis, jjs = _assembly_indices(k)
        v = lg[rows, cols] + float(be3[0])
        adj[bs, iis, jjs] = v
        adj[bs, jjs, iis] = v
    return adj


# revision 28
# speedup vs baseline: 1.6139x; 1.0860x over previous
"""Trainium2 Bass kernel for nn_Decoder (GNN edge decoder).

Math: node MLP -> per-pair edge MLP -> symmetric adjacency.

Key rewrites vs the naive pair loop:
  1. Edge layer-1: concat(z_i, z_j) @ We1 == A_i + B_j with
       A = emb @ We1[:E] + be1,  B = emb @ We1[E:].
  2. relu(A+B) == max(A, -B) + B. The max is ONE vector op (no separate
     relu), and the +B term is linear so it folds into layer-2 as a second
     accumulating matmul with stationary -We2 and rhs -B (PSUM does the add).
     A tunable fraction of groups (LAM_N) instead materialize t1 = relu(A+B)
     with add (DVE) + relu (GPSIMD) and use a single matmul, trading PE time
     against vector-engine time.
  3. Work is batched in 1536-col groups (3 PSUM banks): one pair-op, 3x1-2
     matmuls, one batched relu2 (ACT with fused be2 bias / DVE tensor_scalar
     add+max), 12 tiny mm3 matmuls (stationary = t2 subchunk, rhs = w3 -> 2
     cols out, LoadStationary is free), logits packed into one PSUM bank per
     21 groups then copied+DMA'd out once.

Device layout (per core, uniform SPMD program, data shifted per core):
  segment s = row pair (16s+2k, 16s+2k+1) for core k; j-window
  [16s+1+2k, 255+2k] streamed as contiguous cols of the 2k-shifted node
  tensor (j > 255 region is zero padding, filtered on host).
Host assembles the symmetric adjacency from per-core logit blocks.
"""

import sys

import numpy as np

if "/opt/trn_rl_repo" not in sys.path:
    sys.path.insert(0, "/opt/trn_rl_repo")

import ml_dtypes

B, LAT, ST, N, E, H = 64, 256, 32, 256, 32, 64
NB = N * B  # 16384 node-major cols (col = n*64 + b)
NSEG = 16
GRP = 1536  # group cols (3 PSUM banks)
SLOT = 24  # psum3 cols per group
SBG = 21  # groups per psum3 bank (superblock)
# input tile column layout
APK_O = NB
W2_O = APK_O + NSEG * B  # 17408
W2N_O = W2_O + 128
W3_O = W2N_O + 128
BE2_O = W3_O + 2
INPC = BE2_O + 1  # 17667

# --- engine assignment tuning ---------------------------------------------
# Pool supports only tensor+scalar encodings (no TensorTensor/STT), so the
# pair op is DVE-only; Pool takes relu1 of ARCH-1 groups.
LAM_N = 28  # groups using add+relu1 + single matmul (ARCH-1)
ACT_N = 96  # relu2 on ACT (rest on DVE)

BF16 = ml_dtypes.bfloat16

_cache = {}

# ablation switches for timeline-sim profiling (all False in production)
_abl = {
    "skip_mm3": False,
    "skip_pair": False,
    "skip_relu2": False,
    "mm3_from_m1": False,
    "relu2_from_probe": False,
    "skip_mm2": False,
}


def _groups():
    """(g, s, c0, gF) for each group; segment s window = (255-16s) blocks."""
    if "groups" in _cache:
        return _cache["groups"]
    out = []
    g = 0
    for s in range(NSEG):
        ncols = (255 - 16 * s) * B
        for c0 in range(0, ncols, GRP):
            out.append((g, s, c0, min(GRP, ncols - c0)))
            g += 1
    _cache["groups"] = out
    return out


def _ngrp():
    return len(_groups())


def _spread(n, total):
    """n indices evenly spread in range(total)."""
    if n <= 0:
        return set()
    return {int(i * total / n + total / (2 * n)) for i in range(n)}


def _assign():
    """Per-group (arch1_mode, relu2_on_act) assignment."""
    if "assign" in _cache:
        return _cache["assign"]
    ngrp = _ngrp()
    arch1 = _spread(LAM_N, ngrp)
    act = _spread(ACT_N, ngrp)
    out = [(g in arch1, g in act) for g in range(ngrp)]
    _cache["assign"] = out
    return out


def _build_nc():
    import concourse.bass as bass
    import concourse.mybir as mybir
    from concourse.tile import TileContext

    bf = mybir.dt.bfloat16
    f32 = mybir.dt.float32
    Relu = mybir.ActivationFunctionType.Relu
    Alu = mybir.AluOpType
    nc = bass.Bass()
    ngrp = _ngrp()
    nsb = (ngrp + SBG - 1) // SBG
    lout = nsb * SBG * SLOT
    inp_d = nc.dram_tensor("inp", [128, INPC], bf, kind="ExternalInput")
    out_d = nc.dram_tensor("logits", [128, lout], f32, kind="ExternalOutput")
    assign = _assign()

    with TileContext(nc) as tc:
        with (
            tc.tile_pool(name="const", bufs=1) as cpool,
            tc.tile_pool(name="mw", bufs=4) as mpool,
            tc.tile_pool(name="tw", bufs=4) as tpool,
            tc.tile_pool(name="ps2", bufs=2, space="PSUM") as ps2pool,
            tc.tile_pool(name="ps3", bufs=2, space="PSUM") as ps3pool,
        ):
            inp = cpool.tile([128, INPC], bf, tag="inp")
            # staged input DMA in consumption order: weights+apk first, then
            # bneg pieces, so early groups start while the rest streams in.
            nc.sync.dma_start(inp[:, APK_O:INPC], inp_d[:, APK_O:INPC])
            NPC = 8
            per = ((NB + NPC - 1) // NPC + 63) // 64 * 64
            for pc in range(NPC):
                lo, hi = pc * per, min(NB, (pc + 1) * per)
                if lo < hi:
                    nc.sync.dma_start(inp[:, lo:hi], inp_d[:, lo:hi])
            bneg = inp[:, 0:NB]
            w2 = inp[:, W2_O : W2_O + 128]
            w2n = inp[:, W2N_O : W2N_O + 128]
            w3 = inp[:, W3_O : W3_O + 2]
            be2f = cpool.tile([128, 1], f32, tag="be2f")
            nc.vector.tensor_copy(be2f[:], inp[:, BE2_O : BE2_O + 1])
            be2 = be2f[:]

            # Absorb the HW-DGE queue-semaphore waits of the input DMAs on
            # plain copy instructions (TensorTensor's 3D encoding has too few
            # wait-command slots; see _legalize_sync).
            probe = cpool.tile([128, 8], bf, tag="probe")
            nc.vector.tensor_copy(probe[:, 0:2], inp[:, 0:2])
            nc.gpsimd.tensor_copy(probe[:, 2:4], inp[:, 0:2])
            ps_probe = ps2pool.tile([128, GRP], f32, tag="ps2")
            nc.tensor.matmul(
                ps_probe[:2, :2], inp[:, 0:2], inp[:, 2:4], start=True, stop=True
            )
            nc.scalar.activation(probe[:2, 4:6], ps_probe[:2, :2], Relu)

            groups = _groups()
            # software-pipelined emission: pair(g+1) | mm2(g) | relu2(g-1) |
            # mm3(g-2) so no engine's in-order stream waits on a same-group
            # producer that hasn't had a full stage to finish.
            rhs1s = {}
            ps2s = {}
            t2s = {}
            ps3s = {}

            def em_pair(g):
                _, s, c0, gF = groups[g]
                arch1, _ = assign[g]
                base = (16 * s + 1) * B + c0
                nblk = gF // B
                a_blk = inp[:, APK_O + s * B : APK_O + (s + 1) * B]
                a_bc = bass.AP(
                    a_blk.tensor,
                    a_blk.offset,
                    [list(a_blk.ap[0]), [0, nblk], [1, B]],
                )
                b_sl = inp[:, base : base + gF]
                if _abl["skip_pair"]:
                    rhs1s[g] = b_sl
                    return
                m1 = mpool.tile([128, GRP], bf, tag="m1")
                if arch1:
                    # t1 = relu(A + B) = relu(A - (-B)); relu on Pool
                    nc.vector.tensor_sub(m1[:, :gF], a_bc, b_sl)
                    t1 = tpool.tile([128, GRP], bf, tag="t1")
                    nc.gpsimd.tensor_scalar_max(t1[:, :gF], m1[:, :gF], 0.0)
                    rhs1s[g] = t1
                else:
                    # m1 = max(A, -B); +B folds into the second matmul
                    nc.vector.tensor_max(m1[:, :gF], b_sl, a_bc)
                    rhs1s[g] = m1

            def em_mm2(g):
                _, s, c0, gF = groups[g]
                arch1, _ = assign[g]
                base = (16 * s + 1) * B + c0
                b_sl = inp[:, base : base + gF]
                rhs1 = rhs1s.pop(g)
                if _abl["skip_mm2"]:
                    ps2s[g] = None
                    return
                ps2 = ps2pool.tile([128, GRP], f32, tag="ps2")
                ps2s[g] = ps2
                for ci in range((gF + 511) // 512):
                    o = ci * 512
                    F = min(512, gF - o)
                    if arch1:
                        nc.tensor.matmul(
                            ps2[:, o : o + F],
                            w2,
                            rhs1[:, o : o + F],
                            start=True,
                            stop=True,
                        )
                    else:
                        nc.tensor.matmul(
                            ps2[:, o : o + F],
                            w2,
                            rhs1[:, o : o + F],
                            start=True,
                            stop=False,
                        )
                        nc.tensor.matmul(
                            ps2[:, o : o + F],
                            w2n,
                            b_sl[:, o : o + F],
                            start=False,
                            stop=True,
                        )

            def em_relu2(g):
                _, _, _, gF = groups[g]
                _, on_act = assign[g]
                ps2 = ps2s.pop(g)
                if _abl["skip_relu2"]:
                    t2s[g] = None
                    return
                if _abl["relu2_from_probe"]:
                    ps2 = ps_probe
                t2 = tpool.tile([128, GRP], bf, tag="t2")
                t2s[g] = t2
                if on_act:
                    nc.scalar.activation(t2[:, :gF], ps2[:, :gF], Relu, bias=be2)
                else:
                    nc.vector.tensor_scalar(
                        t2[:, :gF], ps2[:, :gF], be2, 0.0, Alu.add, Alu.max
                    )

            def em_mm3(g):
                _, _, _, gF = groups[g]
                t2 = t2s.pop(g)
                if _abl["skip_mm3"] or t2 is None:
                    if g == ngrp - 1:
                        nc.sync.dma_start(out_d[:, 0:1], be2f[:])
                    return
                sb, slot = divmod(g, SBG)
                if slot == 0:
                    ps3s[sb] = ps3pool.tile(
                        [128, SBG * SLOT], f32, tag="ps3", name=f"ps3_{sb}"
                    )
                ps3 = ps3s[sb]
                if _abl["mm3_from_m1"]:
                    t2 = inp[:, 0:GRP]
                for sc in range((gF + 127) // 128):
                    M = min(128, gF - sc * 128)
                    nc.tensor.matmul(
                        ps3[:M, slot * SLOT + 2 * sc : slot * SLOT + 2 * sc + 2],
                        t2[:, sc * 128 : sc * 128 + M],
                        w3,
                        start=True,
                        stop=True,
                    )
                if slot == SBG - 1 or g == ngrp - 1:
                    lsb = tpool.tile([128, SBG * SLOT], f32, tag="lsb")
                    nc.vector.tensor_copy(lsb[:], ps3s.pop(sb)[:])
                    nc.sync.dma_start(
                        out_d[:, sb * SBG * SLOT : (sb + 1) * SBG * SLOT], lsb[:]
                    )

            LEAD = 1  # pair-op lead
            for g in range(ngrp + 3):
                if g == 0:
                    for gg in range(min(LEAD, ngrp)):
                        em_pair(gg)
                if g + LEAD <= ngrp - 1:
                    em_pair(g + LEAD)
                if g <= ngrp - 1:
                    em_mm2(g)
                if 0 <= g - 1 <= ngrp - 1:
                    em_relu2(g - 1)
                if 0 <= g - 2 <= ngrp - 1:
                    em_mm3(g - 2)

    raw = nc.to_json_bytes()
    legal = _legalize_sync(raw)
    nc.to_json_bytes = lambda: legal
    return nc


def _legalize_sync(bir_bytes):
    """Split multi-wait sync_info into single-wait EventSemaphore preludes.

    The walrus build in this container encodes at most one sync-wait command
    per instruction for several ISA structs; Tile emits up to ~9 on the tail
    drain. Semantics are preserved: waits execute in order on the same engine
    ahead of the original instruction.
    """
    import json as _json

    bir = _json.loads(bir_bytes)
    for f in bir["functions"]:
        ctr = [0]
        templates = {}
        for blk in f["blocks"]:
            for ins in blk.get("instructions") or []:
                if ins.get("opcode") == "EventSemaphore":
                    templates.setdefault(ins.get("engine"), ins)
        for blk in f["blocks"]:
            insts = blk.get("instructions")
            if not insts:
                continue
            out = []
            for ins in insts:
                si = ins.get("sync_info") or {}
                waits = si.get("on_wait") or []
                keep = 0 if ins.get("opcode") == "TensorTensor" else 1
                if len(waits) > keep:
                    tpl = templates.get(ins.get("engine"))
                    if tpl is not None:
                        moved = waits[: len(waits) - keep]
                        for w in moved:
                            ctr[0] += 1
                            nw = _json.loads(_json.dumps(tpl))
                            nw["name"] = f"escw_{ctr[0]}"
                            nw["sync_info"] = {"on_update": [], "on_wait": [w]}
                            out.append(nw)
                        si["on_wait"] = waits[len(waits) - keep :]
                out.append(ins)
            blk["instructions"] = out
    return _json.dumps(bir).encode()


def _host_prep(latent_z, stats, W1, b1, W2, b2, We1, be1, We2, be2, We3, be3):
    """Node MLP + A/B decomposition on host (0.5% of total FLOPs)."""
    x = np.concatenate([latent_z, stats], axis=-1).astype(np.float32)
    h = np.maximum(x @ W1 + b1, 0.0)
    emb = (h @ W2 + b2).reshape(B, N, E)
    A = emb @ We1[:E] + be1  # [B, N, H]
    Bm = emb @ We1[E:]  # [B, N, H]
    A_T = np.ascontiguousarray(A.transpose(2, 1, 0).reshape(H, NB))
    Bneg_T = np.ascontiguousarray((-Bm).transpose(2, 1, 0).reshape(H, NB))
    w2blk = np.zeros((128, 128), np.float32)
    w2blk[:H, :H] = We2
    w2blk[H:, H:] = We2
    w3sep = np.zeros((128, 2), np.float32)
    w3sep[:H, 0] = We3[:, 0]
    w3sep[H:, 1] = We3[:, 0]
    be2col = np.concatenate([be2, be2]).reshape(128, 1).astype(np.float32)
    return A_T, Bneg_T, w2blk, w3sep, be2col, be3


def _shifted(T, sh):
    """[64, NB] -> [64, NB] shifted left by sh cols, zero-padded."""
    out = np.zeros((H, NB), np.float32)
    if sh < NB:
        out[:, : NB - sh] = T[:, sh:]
    return out


def _core_input(A_T, Bneg_T, w2blk, w3sep, be2col, k):
    inp = np.zeros((128, INPC), np.float32)
    bsh = _shifted(Bneg_T, 2 * k * B)
    inp[:H, :NB] = bsh
    inp[H:, :NB] = bsh
    for s in range(NSEG):
        i = 16 * s + 2 * k
        inp[:H, APK_O + s * B : APK_O + (s + 1) * B] = A_T[:, i * B : (i + 1) * B]
        if i + 2 <= N:
            inp[H:, APK_O + s * B : APK_O + (s + 1) * B] = A_T[
                :, (i + 1) * B : (i + 2) * B
            ]
    inp[:, W2_O : W2_O + 128] = w2blk
    inp[:, W2N_O : W2N_O + 128] = -w2blk
    inp[:, W3_O : W3_O + 2] = w3sep
    inp[:, BE2_O : BE2_O + 1] = be2col
    return np.ascontiguousarray(inp.astype(BF16))


def _assembly_indices(k):
    """Vectorized mapping logits[m, col] -> (b, i, j) for core k."""
    key = ("asm", k)
    if key in _cache:
        return _cache[key]
    rows, cols, bs, iis, jjs = [], [], [], [], []
    for g, s, c0, gF in _groups():
        sb, slot = divmod(g, SBG)
        i_t = 16 * s + 2 * k
        for sc in range((gF + 127) // 128):
            M = min(128, gF - sc * 128)
            m = np.arange(M)
            cgl = c0 + sc * 128 + m
            cb = cgl // B  # window block: j = 16s+1+cb+2k
            b = cgl % B
            j = 16 * s + 1 + cb + 2 * k
            ocol = sb * SBG * SLOT + slot * SLOT + 2 * sc
            vt = j <= N - 1
            rows.append(m[vt])
            cols.append(np.full(vt.sum(), ocol))
            bs.append(b[vt])
            iis.append(np.full(vt.sum(), i_t))
            jjs.append(j[vt])
            vb = (j <= N - 1) & (cb >= 1)
            rows.append(m[vb])
            cols.append(np.full(vb.sum(), ocol + 1))
            bs.append(b[vb])
            iis.append(np.full(vb.sum(), i_t + 1))
            jjs.append(j[vb])
    out = tuple(np.concatenate(a) for a in (rows, cols, bs, iis, jjs))
    _cache[key] = out
    return out


def kernel(**inputs):
    from concourse.bass_utils import run_bass_kernel_spmd

    inp = {kk: np.asarray(v, np.float32) for kk, v in inputs.items()}
    A_T, Bneg_T, w2blk, w3sep, be2col, be3 = _host_prep(**inp)

    in_maps = [
        {"inp": _core_input(A_T, Bneg_T, w2blk, w3sep, be2col, k)}
        for k in range(8)
    ]

    import time as _time

    nc = _cache.get("nc")
    if nc is None:
        nc = _build_nc()
        _cache["nc"] = nc
    t0 = _time.time()
    res = run_bass_kernel_spmd(nc, in_maps, core_ids=list(range(8)))
    globals()["last_results"] = res
    globals()["last_run_s"] = _time.time() - t0

    adj = np.zeros((B, N, N), np.float32)
    for k in range(8):
        lg = np.asarray(res.results[k]["logits"], np.float32)
        rows, cols, bs, iis, jjs = _assembly_indices(k)
        v = lg[rows, cols] + float(be3[0])
        adj[bs, iis, jjs] = v
        adj[bs, jjs, iis] = v
    return adj
